# revision 11
# baseline (speedup 1.0000x reference)
"""Trainium2 Bass kernel for nn_PointNet (DGCNN-style): 3x dynamic-kNN edge-conv
(K=2 incl self) + global pooling + BatchNorm + 5-layer MLP head.

Data-parallel over events: 8 events/core x 8 cores. Full inputs in, full output out.

Layout: per-event activations transposed [features(part), nodes(free)].
kNN: T[n,m] = G[n,m] - 0.5*||y_m||^2 built in PSUM (rank-1 fp32r-pair bias +
fp32 Gram accumulation); NN = 2nd-largest T per row via DVE max/max_index
(top-1 = self). Neighbor gather via gpsimd ap_gather. Edge MLP: e0=[y,0],
e1=[y,yj-y] folded to y@(Wa-Wb)+yj@Wb (no elementwise subtract). Head computed
redundantly per core after one AllGather of pooled features.
"""
import sys
import numpy as np

sys.path.insert(0, "/opt/trn_rl_repo")

B, N, F, W = 64, 1024, 5, 128
D2 = 768
NCORES = 8
EV = B // NCORES
NEG_SLOPE, EPS = 0.01, 1e-5
NBLK = N // 128
NCH = N // 512

_cache = {}


def _build(n_events=EV, n_layers=3, with_head=True):
    import concourse.mybir as mybir
    import concourse.bass_isa as bass_isa
    from concourse import bacc
    from concourse.tile import TileContext

    dt = mybir.dt
    AF = mybir.ActivationFunctionType
    Alu = mybir.AluOpType

    nc = bacc.Bacc("TRN2", target_bir_lowering=False, debug=False,
                   num_devices=NCORES)

    def din(name, shape, d=dt.float32):
        return nc.dram_tensor(name, shape, d, kind="ExternalInput")

    xt_d = din("xt", [n_events, F, N])
    w1a_d = din("w1a", [F, W])
    w1ab_d = din("w1ab", [F, W])
    w1b_d = din("w1b", [F, W])
    w2_d = din("w2", [W, W])
    b1_d = din("b1c", [W, 1])
    b2_d = din("b2c", [W, 1])
    cwa_d = din("cwa", [2, W, W])
    cwab_d = din("cwab", [2, W, W])
    cwb_d = din("cwb", [2, W, W])
    cb_d = din("cbc", [2, W, 1])
    if with_head:
        linw_d = din("linw", [5, D2, D2])
        linb_d = din("linb", [5, D2])
        outw_d = din("outw", [D2, 1])
        outb_d = din("outb", [1, 1])
        bng_d = din("bng", [D2])
        bnb_d = din("bnb", [D2])
    out_d = nc.dram_tensor("out", [1, B], dt.float32, kind="ExternalOutput")
    gdbg_d = nc.dram_tensor("gdbg", [128, 6 * n_events], dt.float32,
                            kind="ExternalOutput")

    with TileContext(nc) as tc:
        with tc.tile_pool(name="cst", bufs=1) as cst, \
             tc.tile_pool(name="dram", bufs=4, space="DRAM") as dram:
            # ---------- constants / weights ----------
            onesr = cst.tile([1, 128], dt.float32r, tag="onesr")
            nc.vector.memset(onesr.bitcast(dt.float32), -0.5)
            ident = cst.tile([128, 128], dt.float32, tag="ident")
            tmp1 = cst.tile([128, 128], dt.float32, tag="tmp1")
            nc.vector.memset(tmp1, 1.0)
            nc.gpsimd.affine_select(ident, tmp1, pattern=[[-1, 128]], base=0,
                                    channel_multiplier=1,
                                    compare_op=Alu.is_equal, fill=0.0)
            w1a = cst.tile([F, W], dt.float32, tag="w1a")
            w1ab = cst.tile([F, W], dt.float32, tag="w1ab")
            w1b = cst.tile([F, W], dt.float32, tag="w1b")
            w2 = cst.tile([W, W], dt.float32, tag="w2")
            b1c = cst.tile([W, 1], dt.float32, tag="b1c")
            b2c = cst.tile([W, 1], dt.float32, tag="b2c")
            nc.sync.dma_start(out=w1a, in_=w1a_d.ap())
            nc.sync.dma_start(out=w1ab, in_=w1ab_d.ap())
            nc.sync.dma_start(out=w1b, in_=w1b_d.ap())
            nc.sync.dma_start(out=w2, in_=w2_d.ap())
            nc.sync.dma_start(out=b1c, in_=b1_d.ap())
            nc.sync.dma_start(out=b2c, in_=b2_d.ap())
            cwa, cwab, cwb, cbc = [], [], [], []
            for f in range(2):
                cwa.append(cst.tile([W, W], dt.float32, tag=f"cwa{f}", name=f"cwa{f}"))
                cwab.append(cst.tile([W, W], dt.float32, tag=f"cwab{f}", name=f"cwab{f}"))
                cwb.append(cst.tile([W, W], dt.float32, tag=f"cwb{f}", name=f"cwb{f}"))
                cbc.append(cst.tile([W, 1], dt.float32, tag=f"cbc{f}", name=f"cbc{f}"))
                nc.sync.dma_start(out=cwa[f], in_=cwa_d.ap()[f])
                nc.sync.dma_start(out=cwab[f], in_=cwab_d.ap()[f])
                nc.sync.dma_start(out=cwb[f], in_=cwb_d.ap()[f])
                nc.sync.dma_start(out=cbc[f], in_=cb_d.ap()[f])

            xa = cst.tile([16, N], dt.float32, tag="xa")
            x16 = cst.tile([16, N], dt.float32, tag="x16")
            cm05 = cst.tile([1, N], dt.float32, tag="cm05")
            nc.vector.memset(xa, 0.0)       # rows 0..F-1 overwritten by x DMA
            nc.vector.memset(x16, 0.0)      # rows 0..F by DMAs; rest stay 0
            nc.vector.memset(cm05, -0.5)
            nc.sync.dma_start(out=xa[F:F + 1, :], in_=cm05)  # row F = -0.5
            # squaring xa gives sq+0.25 per column: uniform T shift, argmax-safe

            gown = cst.tile([128, 6 * n_events], dt.float32, tag="gown")
            gv = gown.rearrange("p (c e) -> p c e", e=n_events)

            # ---------- conv phase ----------
            with tc.tile_pool(name="sb", bufs=2) as sb, \
                 tc.tile_pool(name="sb3", bufs=3) as sb3, \
                 tc.tile_pool(name="scan", bufs=2) as scan, \
                 tc.tile_pool(name="pP", bufs=2, space="PSUM") as pP, \
                 tc.tile_pool(name="pM", bufs=1, space="PSUM") as pM:

                def scan_and_gather(make_T, gsrc, chans, tagp):
                    """make_T(a, ps) fills ps [128, N] with T row-block a.
                    Returns gathered neighbor tile [chans, N] (gsrc [chans, N])."""
                    jall = scan.tile([128, NBLK, 8], dt.uint32, tag="jall")
                    vall = scan.tile([128, NBLK, 8], dt.float32, tag="vall")
                    for a in range(NBLK):
                        ps = pP.tile([128, N], dt.float32, tag="psT")
                        make_T(a, ps)
                        nc.vector.max(out=vall[:, a, :], in_=ps)
                        nc.vector.max_index(out=jall[:, a, :],
                                            in_max=vall[:, a, :], in_values=ps)
                    jc16 = scan.tile([128, NBLK], dt.int16, tag="jc16")
                    nc.vector.tensor_copy(jc16, jall[:, :, 1])
                    # wrap via dram bounce: jlin[n] = NN(n), n = a*128 + p
                    jlin = dram.tile([1, N], dt.int16, tag="jlin")
                    nc.sync.dma_start(
                        out=jlin.rearrange("o (p a) -> (o p) a", p=128),
                        in_=jc16)
                    jw = scan.tile([128, N // 16], dt.int16, tag="jw")
                    ngrp = max(1, chans // 16)
                    for g in range(ngrp):
                        nc.sync.dma_start(
                            out=jw[16 * g:16 * (g + 1), :].rearrange(
                                "q (b t) -> q b t", t=8),
                            in_=jlin.rearrange("o (t q b) -> (o q) b t",
                                               t=8, q=16))
                    yj = sb.tile([chans, N], dt.float32, tag="yj" + tagp)
                    nc.gpsimd.ap_gather(
                        out_ap=yj.rearrange("p (n d) -> p n d", d=1),
                        in_ap=gsrc.rearrange("p (n d) -> p n d", d=1),
                        idxs_ap=jw[:chans, :],
                        channels=chans, num_elems=N, d=1, num_idxs=N)
                    return yj

                def pool_layer(yT, ev, lay):
                    nc.vector.tensor_reduce(
                        out=gv[:, lay:lay + 1, ev:ev + 1], in_=yT,
                        axis=mybir.AxisListType.X, op=Alu.add)
                    nc.vector.tensor_reduce(
                        out=gv[:, 3 + lay:4 + lay, ev:ev + 1], in_=yT,
                        axis=mybir.AxisListType.X, op=Alu.max)

                for ev in range(n_events):
                    # ---- L1 ----
                    nc.sync.dma_start(out=xa[:F, :], in_=xt_d.ap()[ev])
                    nc.sync.dma_start(out=x16[:F, :], in_=xt_d.ap()[ev])
                    xsq = sb.tile([16, N], dt.float32, tag="xsq")
                    nc.vector.tensor_tensor(out=xsq, in0=xa, in1=xa,
                                            op=Alu.mult)
                    sqb = sb.tile([16, N], dt.float32, tag="sqb16")
                    nc.gpsimd.partition_all_reduce(
                        out_ap=sqb, in_ap=xsq, channels=16,
                        reduce_op=bass_isa.ReduceOp.add)
                    nc.sync.dma_start(out=x16[F:F + 1, :], in_=sqb[0:1, :])

                    def mkT1(a, ps):
                        for c in range(NCH):
                            nc.tensor.matmul(
                                ps[:, 512 * c:512 * (c + 1)],
                                xa[:F + 1, 128 * a:128 * (a + 1)],
                                x16[:F + 1, 512 * c:512 * (c + 1)],
                                start=True, stop=True)

                    xj = scan_and_gather(mkT1, x16, 16, "L1")
                    pre0 = pM.tile([128, N], dt.float32, tag="pre0")
                    pre1 = pM.tile([128, N], dt.float32, tag="pre1")
                    for c in range(NCH):
                        sl = slice(512 * c, 512 * (c + 1))
                        nc.tensor.matmul(pre0[:, sl], w1a, x16[:F, sl],
                                         start=True, stop=True)
                        nc.tensor.matmul(pre1[:, sl], w1ab, x16[:F, sl],
                                         start=True, stop=False)
                        nc.tensor.matmul(pre1[:, sl], w1b, xj[:F, sl],
                                         start=False, stop=True)
                    h0 = sb3.tile([128, N], dt.float32, tag="h0")
                    h1 = sb3.tile([128, N], dt.float32, tag="h1")
                    nc.scalar.activation(h0, pre0, AF.Relu, bias=b1c, scale=1.0)
                    nc.scalar.activation(h1, pre1, AF.Relu, bias=b1c, scale=1.0)
                    q0 = pM.tile([128, N], dt.float32, tag="pre0")
                    q1 = pM.tile([128, N], dt.float32, tag="pre1")
                    for c in range(NCH):
                        sl = slice(512 * c, 512 * (c + 1))
                        nc.tensor.matmul(q0[:, sl], w2, h0[:, sl],
                                         start=True, stop=True)
                        nc.tensor.matmul(q1[:, sl], w2, h1[:, sl],
                                         start=True, stop=True)
                    r0 = sb3.tile([128, N], dt.float32, tag="h0")
                    r1 = sb3.tile([128, N], dt.float32, tag="h1")
                    nc.scalar.activation(r0, q0, AF.Relu, bias=b2c, scale=1.0)
                    nc.scalar.activation(r1, q1, AF.Relu, bias=b2c, scale=1.0)
                    yT = sb3.tile([128, N], dt.float32, tag="yT")
                    nc.vector.tensor_tensor(out=yT, in0=r0, in1=r1, op=Alu.max)
                    pool_layer(yT, ev, 0)

                    # ---- L2, L3 ----
                    for lay in range(1, n_layers):
                        f = lay - 1
                        ysq = sb.tile([128, N], dt.float32, tag="ysq")
                        nc.vector.tensor_tensor(out=ysq, in0=yT, in1=yT,
                                                op=Alu.mult)
                        sqbc = sb.tile([128, N], dt.float32, tag="sqbc")
                        nc.gpsimd.partition_all_reduce(
                            out_ap=sqbc, in_ap=ysq, channels=128,
                            reduce_op=bass_isa.ReduceOp.add)
                        sqh = sb.tile([1, N], dt.float32r, tag="sqh")
                        sql = sb.tile([1, N], dt.float32r, tag="sql")
                        nc.vector.tensor_copy(sqh, sqbc[0:1, :])
                        nc.vector.tensor_tensor(out=sql, in0=sqbc[0:1, :],
                                                in1=sqh.bitcast(dt.float32),
                                                op=Alu.subtract)
                        ycur = yT

                        def mkT(a, ps, ycur=ycur, sqh=sqh, sql=sql):
                            for c in range(NCH):
                                sl = slice(512 * c, 512 * (c + 1))
                                nc.tensor.matmul(ps[:, sl], onesr, sqh[:, sl],
                                                 start=True, stop=False)
                                nc.tensor.matmul(ps[:, sl], onesr, sql[:, sl],
                                                 start=False, stop=False)
                                nc.tensor.matmul(
                                    ps[:, sl], ycur[:, 128 * a:128 * (a + 1)],
                                    ycur[:, sl], start=False, stop=True)

                        yj = scan_and_gather(mkT, ycur, 128, f"L{lay}")
                        pre0 = pM.tile([128, N], dt.float32, tag="pre0")
                        pre1 = pM.tile([128, N], dt.float32, tag="pre1")
                        for c in range(NCH):
                            sl = slice(512 * c, 512 * (c + 1))
                            nc.tensor.matmul(pre0[:, sl], cwa[f], ycur[:, sl],
                                             start=True, stop=True)
                            nc.tensor.matmul(pre1[:, sl], cwab[f], ycur[:, sl],
                                             start=True, stop=False)
                            nc.tensor.matmul(pre1[:, sl], cwb[f], yj[:, sl],
                                             start=False, stop=True)
                        r0 = sb3.tile([128, N], dt.float32, tag="h0")
                        r1 = sb3.tile([128, N], dt.float32, tag="h1")
                        nc.scalar.activation(r0, pre0, AF.Relu, bias=cbc[f],
                                             scale=1.0)
                        nc.scalar.activation(r1, pre1, AF.Relu, bias=cbc[f],
                                             scale=1.0)
                        yT = sb3.tile([128, N], dt.float32, tag="yT")
                        nc.vector.tensor_tensor(out=yT, in0=r0, in1=r1,
                                                op=Alu.max)
                        pool_layer(yT, ev, lay)

                # mean part: scale by 1/N
                nc.vector.tensor_scalar(out=gv[:, 0:3, :], in0=gv[:, 0:3, :],
                                        scalar1=1.0 / N, scalar2=None,
                                        op0=Alu.mult)
                nc.sync.dma_start(out=gdbg_d.ap(), in_=gown)

            # ---------- head ----------
            if with_head:
                cc_in = dram.tile([128, 6 * n_events], dt.float32, tag="cc_in")
                cc_out = dram.tile([NCORES * 128, 6 * n_events], dt.float32,
                                   tag="cc_out")
                nc.sync.dma_start(out=cc_in, in_=gown)
                nc.gpsimd.collective_compute(
                    "AllGather", Alu.bypass,
                    replica_groups=[list(range(NCORES))],
                    ins=[cc_in.opt()], outs=[cc_out.opt()])

                with tc.tile_pool(name="pH", bufs=2, space="PSUM") as pH:
                    gT = cst.tile([128, 6 * B], dt.float32, tag="gT")
                    gTv = gT.rearrange("p (c e) -> p c e", e=B)
                    ccv = cc_out.rearrange("(r p) (c e) -> r p c e", p=128,
                                           e=n_events)
                    for co in range(NCORES):
                        nc.sync.dma_start(
                            out=gTv[:, :, co * n_events:(co + 1) * n_events],
                            in_=ccv[co])
                    mu = cst.tile([128, 6], dt.float32, tag="mu")
                    nc.vector.tensor_reduce(out=mu, in_=gTv,
                                            axis=mybir.AxisListType.X, op=Alu.add)
                    nc.vector.tensor_scalar(out=mu, in0=mu, scalar1=1.0 / B,
                                            scalar2=None, op0=Alu.mult)
                    dall = cst.tile([128, 6 * B], dt.float32, tag="dall")
                    dv = dall.rearrange("p (c e) -> p c e", e=B)
                    for c in range(6):
                        nc.vector.tensor_scalar(
                            out=dv[:, c, :], in0=gTv[:, c, :],
                            scalar1=mu[:, c:c + 1], scalar2=None,
                            op0=Alu.subtract)
                    dsq = cst.tile([128, 6 * B], dt.float32, tag="dsq")
                    nc.vector.tensor_tensor(out=dsq, in0=dall, in1=dall,
                                            op=Alu.mult)
                    var = cst.tile([128, 6], dt.float32, tag="var")
                    nc.vector.tensor_reduce(
                        out=var, in_=dsq.rearrange("p (c e) -> p c e", e=B),
                        axis=mybir.AxisListType.X, op=Alu.add)
                    epsc = cst.tile([128, 1], dt.float32, tag="epsc")
                    nc.vector.memset(epsc, EPS)
                    std = cst.tile([128, 6], dt.float32, tag="std")
                    nc.scalar.activation(std, var, AF.Sqrt, bias=epsc,
                                         scale=1.0 / B)
                    rstd = cst.tile([128, 6], dt.float32, tag="rstd")
                    nc.vector.reciprocal(rstd, std)
                    bngc = cst.tile([128, 6], dt.float32, tag="bngc")
                    bnbc = cst.tile([128, 6], dt.float32, tag="bnbc")
                    nc.sync.dma_start(
                        out=bngc, in_=bng_d.ap().rearrange("(c p) -> p c", p=128))
                    nc.sync.dma_start(
                        out=bnbc, in_=bnb_d.ap().rearrange("(c p) -> p c", p=128))
                    sc = cst.tile([128, 6], dt.float32, tag="sc")
                    nc.vector.tensor_tensor(out=sc, in0=rstd, in1=bngc,
                                            op=Alu.mult)
                    tcb = cst.tile([128, 6], dt.float32, tag="tcb")
                    nc.vector.tensor_tensor(out=tcb, in0=mu, in1=sc, op=Alu.mult)
                    nc.vector.tensor_tensor(out=tcb, in0=bnbc, in1=tcb,
                                            op=Alu.subtract)
                    h = cst.tile([128, 6 * B], dt.float32, tag="hcur")
                    hv = h.rearrange("p (c e) -> p c e", e=B)
                    for c in range(6):
                        nc.vector.tensor_scalar(
                            out=hv[:, c, :], in0=gTv[:, c, :],
                            scalar1=sc[:, c:c + 1], scalar2=tcb[:, c:c + 1],
                            op0=Alu.mult, op1=Alu.add)
                    wl = cst.tile([128, 6 * D2], dt.float32, tag="wl")
                    wlv = wl.rearrange("p (k o) -> p k o", o=D2)
                    lbc = cst.tile([128, 6], dt.float32, tag="lbc")
                    for lay in range(5):
                        nc.sync.dma_start(
                            out=wlv,
                            in_=linw_d.ap()[lay].rearrange("(k p) o -> p k o",
                                                           p=128))
                        nc.sync.dma_start(
                            out=lbc,
                            in_=linb_d.ap()[lay].rearrange("(c p) -> p c",
                                                           p=128))
                        hn = cst.tile([128, 6 * B], dt.float32,
                                      tag=f"hnext{lay % 2}")
                        hnv = hn.rearrange("p (c e) -> p c e", e=B)
                        for oc in range(6):
                            po = pH.tile([128, B], dt.float32, tag="po")
                            for kc in range(6):
                                nc.tensor.matmul(
                                    po, wlv[:, kc, 128 * oc:128 * (oc + 1)],
                                    hv[:, kc, :], start=(kc == 0),
                                    stop=(kc == 5))
                            nc.scalar.activation(hnv[:, oc, :], po, AF.Lrelu,
                                                 bias=lbc[:, oc:oc + 1],
                                                 scale=1.0, alpha=NEG_SLOPE)
                        h, hv = hn, hnv
                    owc = cst.tile([128, 6], dt.float32, tag="owc")
                    nc.sync.dma_start(
                        out=owc,
                        in_=outw_d.ap().rearrange("(k p) o -> p (k o)", p=128))
                    obc = cst.tile([1, 1], dt.float32, tag="obc")
                    nc.sync.dma_start(out=obc, in_=outb_d.ap())
                    pf = pH.tile([1, B], dt.float32, tag="pf")
                    for kc in range(6):
                        nc.tensor.matmul(pf, owc[:, kc:kc + 1], hv[:, kc, :],
                                         start=(kc == 0), stop=(kc == 5))
                    outs = cst.tile([1, B], dt.float32, tag="outs")
                    nc.vector.tensor_scalar(out=outs, in0=pf,
                                            scalar1=obc[0:1, 0:1], scalar2=None,
                                            op0=Alu.add)
                    nc.sync.dma_start(out=out_d.ap(), in_=outs)
            else:
                zo = cst.tile([1, B], dt.float32, tag="outs")
                nc.vector.memset(zo, 0.0)
                nc.sync.dma_start(out=out_d.ap(), in_=zo)

    nc.compile()
    return nc


def _prep_inputs(inputs):
    x = np.ascontiguousarray(np.asarray(inputs["x"], np.float32))
    p1_w1 = np.asarray(inputs["p1_w1"], np.float32)
    W1a, W1b = p1_w1[:F], p1_w1[F:]
    c_w = np.asarray(inputs["c_w"], np.float32)
    cwa, cwb = c_w[:, :W, :], c_w[:, W:, :]
    shared = {
        "w1a": np.ascontiguousarray(W1a),
        "w1ab": np.ascontiguousarray(W1a - W1b),
        "w1b": np.ascontiguousarray(W1b),
        "w2": np.asarray(inputs["p1_w2"], np.float32),
        "b1c": np.asarray(inputs["p1_b1"], np.float32).reshape(W, 1),
        "b2c": np.asarray(inputs["p1_b2"], np.float32).reshape(W, 1),
        "cwa": np.ascontiguousarray(cwa),
        "cwab": np.ascontiguousarray(cwa - cwb),
        "cwb": np.ascontiguousarray(cwb),
        "cbc": np.ascontiguousarray(
            np.asarray(inputs["c_b"], np.float32).reshape(2, W, 1)),
        "linw": np.asarray(inputs["lin_w"], np.float32),
        "linb": np.asarray(inputs["lin_b"], np.float32),
        "outw": np.asarray(inputs["out_w"], np.float32),
        "outb": np.asarray(inputs["out_b"], np.float32).reshape(1, 1),
        "bng": np.asarray(inputs["bn_g"], np.float32),
        "bnb": np.asarray(inputs["bn_b"], np.float32),
    }
    in_maps = []
    for c in range(NCORES):
        xs = x[c * EV:(c + 1) * EV]
        m = dict(shared)
        m["xt"] = np.ascontiguousarray(xs.transpose(0, 2, 1))
        in_maps.append(m)
    return in_maps


def kernel(**inputs) -> np.ndarray:
    from concourse import bass_utils
    if "nc" not in _cache:
        _cache["nc"] = _build()
    nc = _cache["nc"]
    in_maps = _prep_inputs(inputs)
    res = bass_utils.run_bass_kernel_spmd(nc, in_maps,
                                          core_ids=list(range(NCORES)))
    _cache["last_results"] = res
    return np.asarray(res.results[0]["out"]).reshape(B)
